# revision 18
# baseline (speedup 1.0000x reference)
"""AdjustedNonLocalBlock on 8 TRN2 NeuronCores (float32r pipeline).

Math (per batch, N = H*W = 4096 positions):
    f = theta(x1)^T phi(x0);  P = softmax(f, axis=-1);
    y = P @ g(x0)^T;  out = W_w y^T + W_b + x0.

Reductions:
  - f[q,k] = x1[:,q]^T A x0[:,k] + t3[k] (+ per-q consts, dropped --
    softmax-invariant), A = theta_w^T phi_w, t3 = (phi_w^T theta_b)^T x0.
    t3 enters as the per-partition bias of the exp activation.
  - g's bias folds into the output bias b_out = W_w g_b + W_b (softmax
    rows sum to 1); the 1/Z normalization commutes with W_w (per-query
    diagonal), applied between the attention and projection matmuls.
  - Z comes free as a ones-column appended to g in the second matmul.

Sharding: core i = (batch i//2, query half i%2); softmax is over keys,
fully core-local -> no collectives.

Per-core dataflow (float32r matmuls, fp32 PSUM):
  U = A @ X0 [128,4096]; gaug per key tile kt: [t3 | g_raw^T (64) | 1].
  For each 1024-query pair qp (2 of them):
    for kt (32): S[128k,1024q] = U_kt^T X1_qp (two [128,512] matmuls)
      E = exp(S + t3_kt) [128,1024] on ScalarE (per-partition bias)
      Ya/Yb[65,512] += [g|1]^T E-half (two matmuls, separate banks).
    Z = Y[64]; y = Y[0:64] * (1/Z broadcast by a ones-matmul);
    out = W_aug @ [y; 1] + x0_res.
  U/gaug production is emitted just-in-time inside the first qp pass;
  the qp0 epilogues run inside qp1's loop (the DVE reciprocal runs
  early, the TensorE part late) so the PE never stalls on them; only
  the last pair of epilogues is exposed in the tail.
"""

import numpy as np

import concourse.bacc as bacc
import concourse.mybir as mybir
import concourse.tile as tile
from concourse.bass_utils import run_bass_kernel_spmd

B, C, CI = 4, 128, 64
H, W = 64, 64
N = H * W              # 4096
NCORES = 8
QH = N // 2            # 2048 queries per core
KT = N // 128          # 32 key tiles of 128
GSTR = 66              # gaug per-tile layout: [t3 | g (64) | ones]

F32 = mybir.dt.float32
F32R = mybir.dt.float32r

_CACHE = {}


def _f32(ap):
    return ap.bitcast(F32)


def _build():
    if "nc" in _CACHE:
        return _CACHE["nc"]

    nc = bacc.Bacc("TRN2", target_bir_lowering=False, debug=False,
                   num_devices=NCORES)
    x0_ext = nc.declare_dram_parameter("x0", [C, N], F32R, isOutput=False)
    x1_ext = nc.declare_dram_parameter("x1h", [C, QH], F32R, isOutput=False)
    res_ext = nc.declare_dram_parameter("res", [C, QH], F32, isOutput=False)
    at_ext = nc.declare_dram_parameter("a_t", [C, C], F32R, isOutput=False)
    gv_ext = nc.declare_dram_parameter("gv", [C, CI + 2], F32R,
                                       isOutput=False)
    wa_ext = nc.declare_dram_parameter("w_aug", [CI + 1, C], F32R,
                                       isOutput=False)
    eye_ext = nc.declare_dram_parameter("eye", [C, C], F32, isOutput=False)
    out_ext = nc.declare_dram_parameter("out", [C, QH], F32, isOutput=True)

    AF = mybir.ActivationFunctionType

    with tile.TileContext(nc) as tc:
        with (
            tc.tile_pool(name="const", bufs=1) as constp,
            tc.tile_pool(name="data", bufs=1) as datap,
            tc.tile_pool(name="epool", bufs=6) as epool,
            tc.tile_pool(name="spool", bufs=2, space="PSUM") as spool,
            tc.tile_pool(name="ypool", bufs=2, space="PSUM") as ypool,
            tc.tile_pool(name="smallp", bufs=2, space="PSUM") as smallp,
            tc.tile_pool(name="ysbp", bufs=2) as ysbp,
            tc.tile_pool(name="outp", bufs=3) as outp,
            tc.tile_pool(name="rzp", bufs=2) as rzp,
        ):
            # table preload: a tiny Exp warms the exp table set while
            # the input DMAs are still in flight
            scr = constp.tile([1, 2], F32)
            nc.vector.memset(scr[:], 1.0)
            nc.scalar.activation(scr[0:1, 1:2], scr[0:1, 0:1], AF.Exp)

            ones_sb = constp.tile([1, CI], F32)
            nc.vector.memset(ones_sb[:], 1.0)

            # small inputs first, then the big ones in chunks so the
            # first compute can start after ~1 chunk
            at_sb = constp.tile([C, C], F32R)
            nc.sync.dma_start(at_sb[:], at_ext[:])
            x0_sb = datap.tile([C, N], F32R)
            x1_sb = datap.tile([C, QH], F32R)
            nc.sync.dma_start(x0_sb[:, 0:512], x0_ext[:, 0:512])
            nc.sync.dma_start(x1_sb[:, 0:512], x1_ext[:, 0:512])
            nc.sync.dma_start(x1_sb[:, 512:1024], x1_ext[:, 512:1024])
            gv_sb = constp.tile([C, CI + 2], F32R)
            nc.sync.dma_start(gv_sb[:], gv_ext[:])
            nc.sync.dma_start(x0_sb[:, 512:1024], x0_ext[:, 512:1024])
            nc.sync.dma_start(x0_sb[:, 1024:2048], x0_ext[:, 1024:2048])
            nc.sync.dma_start(x0_sb[:, 2048:4096], x0_ext[:, 2048:4096])
            wa_sb = constp.tile([CI + 1, C], F32R)
            nc.sync.dma_start(wa_sb[:], wa_ext[:])
            eye_sb = constp.tile([C, C], F32)
            nc.sync.dma_start(eye_sb[:], eye_ext[:])
            id1_sb = constp.tile([1, 1], F32)
            nc.vector.memset(id1_sb[:], 1.0)
            nc.sync.dma_start(x1_sb[:, 1024:2048], x1_ext[:, 1024:2048])
            res_sb = datap.tile([C, QH], F32)

            U_sb = datap.tile([C, N], F32R)
            gaug_sb = datap.tile([C, KT * GSTR], F32R)
            nc.vector.memset(_f32(gaug_sb[:]), 1.0)  # preset ones cols
            yaug_sb = datap.tile([CI + 1, QH], F32R)
            nc.vector.memset(_f32(yaug_sb[CI:CI + 1, :]), 1.0)

            def emit_u_chunk(c):
                pu = smallp.tile([C, 512], F32, tag="sm")
                nc.tensor.matmul(pu[:], at_sb[:],
                                 x0_sb[:, c * 512:(c + 1) * 512],
                                 start=True, stop=True)
                nc.vector.tensor_copy(U_sb[:, c * 512:(c + 1) * 512], pu[:])

            def emit_gaug(kt):
                # [t3 | g_raw^T] -> cols 0..64 of this tile's stripe
                pg = smallp.tile([C, 512], F32, tag="sm")
                nc.tensor.matmul(pg[:, 0:CI + 2],
                                 x0_sb[:, kt * 128:(kt + 1) * 128],
                                 gv_sb[:], start=True, stop=True)
                nc.vector.tensor_copy(
                    gaug_sb[:, kt * GSTR:kt * GSTR + CI + 1], pg[:, 0:CI + 1])

            # prologue pieces needed before the first S tile / mm2
            emit_u_chunk(0)
            emit_gaug(0)
            emit_gaug(1)

            def emit_mm1(qp, kt):
                s = spool.tile([C, 1024], F32)
                q0 = qp * 1024
                lhs = U_sb[:, kt * 128:(kt + 1) * 128]
                nc.tensor.matmul(s[:, 0:512], lhs, x1_sb[:, q0:q0 + 512],
                                 start=True, stop=True)
                nc.tensor.matmul(s[:, 512:1024], lhs,
                                 x1_sb[:, q0 + 512:q0 + 1024],
                                 start=True, stop=True)
                return s

            def emit_epilogue_fronts(items):
                # Per Y: copy (frees the bank), then reciprocal of Z,
                # then GPSIMD broadcasts 1/Z across partitions (keeps
                # both the PE and the mul off the 1/Z critical path).
                ret = []
                for qc, Y in items:
                    ysb = ysbp.tile([CI, 512], F32)
                    nc.vector.tensor_copy(ysb[:], Y[0:CI, :])
                    zrow = rzp.tile([1, 512], F32, tag="zrow")
                    nc.vector.tensor_copy(zrow[:], Y[CI:CI + 1, :])
                    rz = rzp.tile([1, 512], F32)
                    nc.vector.reciprocal(rz[:], zrow[:])
                    bcs = ysbp.tile([CI, 512], F32, tag="bcs")
                    nc.gpsimd.partition_broadcast(bcs[:], rz[:], channels=CI)
                    ret.append((qc, ysb, bcs))
                return ret

            def emit_epilogue_fronts_tail(items):
                # Tail variant: 1/Z computed on Z transposed across 128
                # partitions (PE transposes are free in the tail; the
                # [1,512] DVE reciprocal would cost 3.3us each).
                pre = []
                for qc, Y in items:
                    ysb = ysbp.tile([CI, 512], F32)
                    nc.vector.tensor_copy(ysb[:], Y[0:CI, :])
                    zrow = rzp.tile([1, 512], F32, tag="zrow")
                    nc.vector.tensor_copy(zrow[:], Y[CI:CI + 1, :])
                    pre.append((qc, ysb, zrow))
                ret = []
                for qc, ysb, zrow in pre:
                    zt = smallp.tile([C, 512], F32, tag="sm")
                    for j in range(4):
                        nc.tensor.transpose(
                            zt[:, j:j + 1],
                            zrow[0:1, j * 128:(j + 1) * 128], id1_sb[:])
                    rzt = rzp.tile([C, 4], F32, tag="rzt")
                    nc.vector.reciprocal(rzt[:], zt[:, 0:4])
                    tr2 = smallp.tile([C, 512], F32, tag="sm")
                    for j in range(4):
                        nc.tensor.transpose(tr2[0:1, j * 128:(j + 1) * 128],
                                            rzt[:, j:j + 1], eye_sb[:])
                    rz = rzp.tile([1, 512], F32)
                    nc.vector.tensor_copy(rz[:], tr2[0:1, 0:512])
                    bcs = ysbp.tile([CI, 512], F32, tag="bcs")
                    nc.gpsimd.partition_broadcast(bcs[:], rz[:], channels=CI)
                    ret.append((qc, ysb, bcs))
                return ret

            def emit_epilogue_back(qc, ysb, bcs, anchor=None):
                q0 = qc * 512
                nc.vector.tensor_mul(yaug_sb[0:CI, q0:q0 + 512],
                                     ysb[:], bcs[:])
                pr = smallp.tile([C, 512], F32, tag="sm")
                prj = nc.tensor.matmul(pr[:], wa_sb[:],
                                       yaug_sb[:, q0:q0 + 512],
                                       start=True, stop=True)
                if anchor is not None:
                    # pin the projection behind a late matmul so the
                    # scheduler cannot hoist it into a stall
                    tile.add_dep_helper(prj.ins, anchor.ins, False,
                                        "defer epilogue proj")
                ot = outp.tile([C, 512], F32)
                nc.vector.tensor_add(ot[:], pr[:], res_sb[:, q0:q0 + 512])
                nc.sync.dma_start(out_ext[:, q0:q0 + 512], ot[:])

            fronts = []  # epilogue fronts whose TensorE part is pending
            for qp in range(2):
                ya = ypool.tile([CI + 1, 512], F32, tag="y")
                yb = ypool.tile([CI + 1, 512], F32, tag="y")
                s_cur = emit_mm1(qp, 0)
                for kt in range(KT):
                    e = epool.tile([C, 1024], F32R)
                    nc.scalar.activation(
                        e[:], s_cur[:], AF.Exp,
                        bias=_f32(gaug_sb[:, kt * GSTR:kt * GSTR + 1]))
                    if qp == 0:
                        # just-in-time prologue during the first pass
                        if kt % 4 == 0 and kt // 4 + 1 < 8:
                            emit_u_chunk(kt // 4 + 1)
                        if kt + 2 < KT:
                            emit_gaug(kt + 2)
                        if kt == 0:
                            nc.sync.dma_start(res_sb[:], res_ext[:])
                    else:
                        # TensorE part of qp0's epilogues, far enough in
                        # that the reciprocal results are long ready
                        if kt in (10, 12) and fronts:
                            emit_epilogue_back(*fronts.pop(0),
                                               anchor=prev_mm2)
                    if kt + 1 < KT:
                        s_cur = emit_mm1(qp, kt + 1)
                    elif qp == 0:
                        s_cur = emit_mm1(1, 0)
                    st, sp = kt == 0, kt == KT - 1
                    glhs = gaug_sb[:, kt * GSTR + 1:kt * GSTR + GSTR]
                    prev_mm2 = nc.tensor.matmul(ya[:], glhs, e[:, 0:512],
                                                start=st, stop=sp)
                    nc.tensor.matmul(yb[:], glhs, e[:, 512:1024],
                                     start=st, stop=sp)
                # DVE fronts run now (free the Y banks for the next qp)
                if qp == 0:
                    fronts.extend(emit_epilogue_fronts(
                        [(2 * qp, ya), (2 * qp + 1, yb)]))
                else:
                    fronts.extend(emit_epilogue_fronts_tail(
                        [(2 * qp, ya), (2 * qp + 1, yb)]))

            # exposed tail: fast 1/Z via PE transposes (PE is idle),
            # then the usual backs
            tails = []
            for qc, ysb, bcs in fronts:
                tails.append((qc, ysb, bcs))
            fronts.clear()
            emit_epilogue_back(*tails.pop(0))
            emit_epilogue_back(*tails.pop(0))

    nc.compile()
    _CACHE["nc"] = nc
    return nc


def _prep_in_maps(inputs):
    x0 = np.ascontiguousarray(np.asarray(inputs["x0"], np.float32))
    x1 = np.ascontiguousarray(np.asarray(inputs["x1"], np.float32))
    g_w = np.asarray(inputs["g_w"], np.float32)
    g_b = np.asarray(inputs["g_b"], np.float32)
    theta_w = np.asarray(inputs["theta_w"], np.float32)
    theta_b = np.asarray(inputs["theta_b"], np.float32)
    phi_w = np.asarray(inputs["phi_w"], np.float32)
    W_w = np.asarray(inputs["W_w"], np.float32)
    W_b = np.asarray(inputs["W_b"], np.float32)

    a_t = np.ascontiguousarray(phi_w.T @ theta_w)            # [C, C]
    v = phi_w.T @ theta_b                                    # [C]
    gv = np.ascontiguousarray(np.concatenate(
        [v[:, None], g_w.T, np.zeros((C, 1), np.float32)], axis=1))
    b_out = W_w @ g_b + W_b                                  # [C]
    w_aug = np.ascontiguousarray(
        np.concatenate([W_w.T, b_out[None, :]], axis=0))     # [65, C]

    in_maps = []
    for core in range(NCORES):
        b, hh = core // 2, core % 2
        x0f = x0[b].reshape(C, N)
        x1f = x1[b].reshape(C, N)
        in_maps.append({
            "x0": x0f,
            "x1h": np.ascontiguousarray(x1f[:, hh * QH:(hh + 1) * QH]),
            "res": np.ascontiguousarray(x0f[:, hh * QH:(hh + 1) * QH]),
            "a_t": a_t,
            "gv": gv,
            "w_aug": w_aug,
            "eye": np.eye(C, dtype=np.float32),
        })
    return in_maps


def _run(inputs, trace=False):
    nc = _build()
    in_maps = _prep_in_maps(inputs)
    res = run_bass_kernel_spmd(nc, in_maps, core_ids=list(range(NCORES)),
                               trace=trace)
    out = np.empty((B, C, N), np.float32)
    for core in range(NCORES):
        b, hh = core // 2, core % 2
        out[b][:, hh * QH:(hh + 1) * QH] = res.results[core]["out"]
    return out.reshape(B, C, H, W), res


def kernel(**inputs) -> np.ndarray:
    out, _ = _run(inputs, trace=False)
    return out


# revision 19
# speedup vs baseline: 1.1801x; 1.1801x over previous
"""AdjustedNonLocalBlock on 8 TRN2 NeuronCores (float32r pipeline).

Math (per batch, N = H*W = 4096 positions):
    f = theta(x1)^T phi(x0);  P = softmax(f, axis=-1);
    y = P @ g(x0)^T;  out = W_w y^T + W_b + x0.

Reductions:
  - f[q,k] = x1[:,q]^T A x0[:,k] + t3[k] (+ per-q consts, dropped --
    softmax-invariant), A = theta_w^T phi_w, t3 = (phi_w^T theta_b)^T x0.
    t3 enters as the per-partition bias of the exp activation.
  - g's bias folds into the output bias b_out = W_w g_b + W_b (softmax
    rows sum to 1); the 1/Z normalization commutes with W_w (per-query
    diagonal), applied between the attention and projection matmuls.
  - Z comes free as a ones-column appended to g in the second matmul.

Sharding: core i = (batch i//2, query half i%2); softmax is over keys,
fully core-local -> no collectives.

Per-core dataflow (float32r matmuls, fp32 PSUM):
  U = A @ X0 [128,4096]; gaug per key tile kt: [t3 | g_raw^T (64) | 1].
  For each 1024-query pair qp (2 of them):
    for kt (32): S[128k,1024q] = U_kt^T X1_qp (two [128,512] matmuls)
      E = exp(S + t3_kt) [128,1024] on ScalarE (per-partition bias)
      Ya/Yb[65,512] += [g|1]^T E-half (two matmuls, separate banks).
    Z = Y[64]; y = Y[0:64] * (1/Z broadcast by a ones-matmul);
    out = W_aug @ [y; 1] + x0_res.
  U/gaug production is emitted just-in-time inside the first qp pass;
  the qp0 epilogues run inside qp1's loop (the DVE reciprocal runs
  early, the TensorE part late) so the PE never stalls on them; only
  the last pair of epilogues is exposed in the tail.
"""

import numpy as np

import concourse.bacc as bacc
import concourse.mybir as mybir
import concourse.tile as tile
from concourse.bass_utils import run_bass_kernel_spmd

B, C, CI = 4, 128, 64
H, W = 64, 64
N = H * W              # 4096
NCORES = 8
QH = N // 2            # 2048 queries per core
KT = N // 128          # 32 key tiles of 128
GSTR = 66              # gaug per-tile layout: [t3 | g (64) | ones]

F32 = mybir.dt.float32
F32R = mybir.dt.float32r

_CACHE = {}


def _f32(ap):
    return ap.bitcast(F32)


def _build():
    if "nc" in _CACHE:
        return _CACHE["nc"]

    nc = bacc.Bacc("TRN2", target_bir_lowering=False, debug=False,
                   num_devices=NCORES)
    x0_ext = nc.declare_dram_parameter("x0", [C, N], F32R, isOutput=False)
    x1_ext = nc.declare_dram_parameter("x1h", [C, QH], F32R, isOutput=False)
    res_ext = nc.declare_dram_parameter("res", [C, QH], F32, isOutput=False)
    at_ext = nc.declare_dram_parameter("a_t", [C, C], F32R, isOutput=False)
    gv_ext = nc.declare_dram_parameter("gv", [C, CI + 2], F32R,
                                       isOutput=False)
    wa_ext = nc.declare_dram_parameter("w_aug", [CI + 1, C], F32R,
                                       isOutput=False)
    eye_ext = nc.declare_dram_parameter("eye", [C, C], F32, isOutput=False)
    out_ext = nc.declare_dram_parameter("out", [C, QH], F32, isOutput=True)

    AF = mybir.ActivationFunctionType

    with tile.TileContext(nc) as tc:
        with (
            tc.tile_pool(name="const", bufs=1) as constp,
            tc.tile_pool(name="data", bufs=1) as datap,
            tc.tile_pool(name="epool", bufs=4) as epool,
            tc.tile_pool(name="spool", bufs=2, space="PSUM") as spool,
            tc.tile_pool(name="ypool", bufs=2, space="PSUM") as ypool,
            tc.tile_pool(name="smallp", bufs=2, space="PSUM") as smallp,
            tc.tile_pool(name="ysbp", bufs=2) as ysbp,
            tc.tile_pool(name="outp", bufs=3) as outp,
            tc.tile_pool(name="rzp", bufs=2) as rzp,
        ):
            # table preload: a tiny Exp warms the exp table set while
            # the input DMAs are still in flight
            scr = constp.tile([1, 2], F32)
            nc.vector.memset(scr[:], 1.0)
            nc.scalar.activation(scr[0:1, 1:2], scr[0:1, 0:1], AF.Exp)

            ones_sb = constp.tile([1, CI], F32)
            nc.vector.memset(ones_sb[:], 1.0)

            # small inputs first, then the big ones in chunks so the
            # first compute can start after ~1 chunk
            at_sb = constp.tile([C, C], F32R)
            nc.sync.dma_start(at_sb[:], at_ext[:])
            x0_sb = datap.tile([C, N], F32R)
            x1_sb = datap.tile([C, QH], F32R)
            nc.sync.dma_start(x0_sb[:, 0:512], x0_ext[:, 0:512])
            nc.sync.dma_start(x1_sb[:, 0:512], x1_ext[:, 0:512])
            nc.sync.dma_start(x1_sb[:, 512:1024], x1_ext[:, 512:1024])
            gv_sb = constp.tile([C, CI + 2], F32R)
            nc.sync.dma_start(gv_sb[:], gv_ext[:])
            nc.sync.dma_start(x0_sb[:, 512:1024], x0_ext[:, 512:1024])
            nc.sync.dma_start(x0_sb[:, 1024:2048], x0_ext[:, 1024:2048])
            nc.sync.dma_start(x0_sb[:, 2048:4096], x0_ext[:, 2048:4096])
            wa_sb = constp.tile([CI + 1, C], F32R)
            nc.sync.dma_start(wa_sb[:], wa_ext[:])
            eye_sb = constp.tile([C, C], F32)
            nc.sync.dma_start(eye_sb[:], eye_ext[:])
            id1_sb = constp.tile([1, 1], F32)
            nc.vector.memset(id1_sb[:], 1.0)
            nc.sync.dma_start(x1_sb[:, 1024:2048], x1_ext[:, 1024:2048])
            res_sb = datap.tile([C, QH], F32)

            U_sb = datap.tile([C, N], F32R)
            gaug_sb = datap.tile([C, KT * GSTR], F32R)
            nc.vector.memset(_f32(gaug_sb[:]), 1.0)  # preset ones cols
            yaug_sb = datap.tile([CI + 1, QH], F32R)
            nc.vector.memset(_f32(yaug_sb[CI:CI + 1, :]), 1.0)

            def emit_u_chunk(c):
                pu = smallp.tile([C, 512], F32, tag="sm")
                nc.tensor.matmul(pu[:], at_sb[:],
                                 x0_sb[:, c * 512:(c + 1) * 512],
                                 start=True, stop=True)
                nc.vector.tensor_copy(U_sb[:, c * 512:(c + 1) * 512], pu[:])

            def emit_gaug(kt):
                # [t3 | g_raw^T] -> cols 0..64 of this tile's stripe
                pg = smallp.tile([C, 512], F32, tag="sm")
                nc.tensor.matmul(pg[:, 0:CI + 2],
                                 x0_sb[:, kt * 128:(kt + 1) * 128],
                                 gv_sb[:], start=True, stop=True)
                nc.vector.tensor_copy(
                    gaug_sb[:, kt * GSTR:kt * GSTR + CI + 1], pg[:, 0:CI + 1])

            # prologue pieces needed before the first S tile / mm2
            emit_u_chunk(0)
            emit_gaug(0)
            emit_gaug(1)

            def emit_mm1(qp, kt):
                s = spool.tile([C, 1024], F32)
                q0 = qp * 1024
                lhs = U_sb[:, kt * 128:(kt + 1) * 128]
                nc.tensor.matmul(s[:, 0:512], lhs, x1_sb[:, q0:q0 + 512],
                                 start=True, stop=True)
                nc.tensor.matmul(s[:, 512:1024], lhs,
                                 x1_sb[:, q0 + 512:q0 + 1024],
                                 start=True, stop=True)
                return s

            def emit_epilogue_fronts(items):
                # Per Y: copy (frees the bank), then reciprocal of Z,
                # then GPSIMD broadcasts 1/Z across partitions (keeps
                # both the PE and the mul off the 1/Z critical path).
                ret = []
                for qc, Y in items:
                    ysb = ysbp.tile([CI, 512], F32)
                    nc.vector.tensor_copy(ysb[:], Y[0:CI, :])
                    zrow = rzp.tile([1, 512], F32, tag="zrow")
                    nc.vector.tensor_copy(zrow[:], Y[CI:CI + 1, :])
                    rz = rzp.tile([1, 512], F32)
                    nc.vector.reciprocal(rz[:], zrow[:])
                    bcs = ysbp.tile([CI, 512], F32, tag="bcs")
                    nc.gpsimd.partition_broadcast(bcs[:], rz[:], channels=CI)
                    ret.append((qc, ysb, bcs))
                return ret

            def emit_epilogue_fronts_tail(items):
                # Tail variant: 1/Z computed on Z transposed across 128
                # partitions (PE transposes are free in the tail; the
                # [1,512] DVE reciprocal would cost 3.3us each).
                pre = []
                for qc, Y in items:
                    ysb = ysbp.tile([CI, 512], F32)
                    nc.vector.tensor_copy(ysb[:], Y[0:CI, :])
                    zrow = rzp.tile([1, 512], F32, tag="zrow")
                    nc.vector.tensor_copy(zrow[:], Y[CI:CI + 1, :])
                    pre.append((qc, ysb, zrow))
                ret = []
                for qc, ysb, zrow in pre:
                    zt = smallp.tile([C, 512], F32, tag="sm")
                    for j in range(4):
                        nc.tensor.transpose(
                            zt[:, j:j + 1],
                            zrow[0:1, j * 128:(j + 1) * 128], id1_sb[:])
                    rzt = rzp.tile([C, 4], F32, tag="rzt")
                    nc.vector.reciprocal(rzt[:], zt[:, 0:4])
                    tr2 = smallp.tile([C, 512], F32, tag="sm")
                    for j in range(4):
                        nc.tensor.transpose(tr2[0:1, j * 128:(j + 1) * 128],
                                            rzt[:, j:j + 1], eye_sb[:])
                    rz = rzp.tile([1, 512], F32)
                    nc.vector.tensor_copy(rz[:], tr2[0:1, 0:512])
                    bcs = ysbp.tile([CI, 512], F32, tag="bcs")
                    nc.gpsimd.partition_broadcast(bcs[:], rz[:], channels=CI)
                    ret.append((qc, ysb, bcs))
                return ret

            def emit_epilogue_back(qc, ysb, bcs, anchor=None):
                q0 = qc * 512
                nc.vector.tensor_mul(yaug_sb[0:CI, q0:q0 + 512],
                                     ysb[:], bcs[:])
                pr = smallp.tile([C, 512], F32, tag="sm")
                prj = nc.tensor.matmul(pr[:], wa_sb[:],
                                       yaug_sb[:, q0:q0 + 512],
                                       start=True, stop=True)
                if anchor is not None:
                    # pin the projection behind a late matmul so the
                    # scheduler cannot hoist it into a stall
                    tile.add_dep_helper(prj.ins, anchor.ins, False,
                                        "defer epilogue proj")
                ot = outp.tile([C, 512], F32)
                nc.vector.tensor_add(ot[:], pr[:], res_sb[:, q0:q0 + 512])
                nc.sync.dma_start(out_ext[:, q0:q0 + 512], ot[:])

            fronts = []  # epilogue fronts whose TensorE part is pending
            for qp in range(2):
                ya = ypool.tile([CI + 1, 512], F32, tag="y")
                yb = ypool.tile([CI + 1, 512], F32, tag="y")
                s_cur = emit_mm1(qp, 0)
                for kt in range(KT):
                    e = epool.tile([C, 1024], F32R)
                    nc.scalar.activation(
                        e[:], s_cur[:], AF.Exp,
                        bias=_f32(gaug_sb[:, kt * GSTR:kt * GSTR + 1]))
                    if qp == 0:
                        # just-in-time prologue during the first pass
                        if kt % 4 == 0 and kt // 4 + 1 < 8:
                            emit_u_chunk(kt // 4 + 1)
                        if kt + 2 < KT:
                            emit_gaug(kt + 2)
                        if kt == 0:
                            nc.sync.dma_start(res_sb[:], res_ext[:])
                    else:
                        # TensorE part of qp0's epilogues, far enough in
                        # that the reciprocal results are long ready
                        if kt in (10, 12) and fronts:
                            emit_epilogue_back(*fronts.pop(0),
                                               anchor=prev_mm2)
                    if kt + 1 < KT:
                        s_cur = emit_mm1(qp, kt + 1)
                    elif qp == 0:
                        s_cur = emit_mm1(1, 0)
                    st, sp = kt == 0, kt == KT - 1
                    glhs = gaug_sb[:, kt * GSTR + 1:kt * GSTR + GSTR]
                    prev_mm2 = nc.tensor.matmul(ya[:], glhs, e[:, 0:512],
                                                start=st, stop=sp)
                    nc.tensor.matmul(yb[:], glhs, e[:, 512:1024],
                                     start=st, stop=sp)
                # DVE fronts run now (free the Y banks for the next qp)
                if qp == 0:
                    fronts.extend(emit_epilogue_fronts(
                        [(2 * qp, ya), (2 * qp + 1, yb)]))
                else:
                    fronts.extend(emit_epilogue_fronts_tail(
                        [(2 * qp, ya), (2 * qp + 1, yb)]))

            # exposed tail: fast 1/Z via PE transposes (PE is idle),
            # then the usual backs
            tails = []
            for qc, ysb, bcs in fronts:
                tails.append((qc, ysb, bcs))
            fronts.clear()
            emit_epilogue_back(*tails.pop(0))
            emit_epilogue_back(*tails.pop(0))

    nc.compile()
    _CACHE["nc"] = nc
    return nc


def _prep_in_maps(inputs):
    x0 = np.ascontiguousarray(np.asarray(inputs["x0"], np.float32))
    x1 = np.ascontiguousarray(np.asarray(inputs["x1"], np.float32))
    g_w = np.asarray(inputs["g_w"], np.float32)
    g_b = np.asarray(inputs["g_b"], np.float32)
    theta_w = np.asarray(inputs["theta_w"], np.float32)
    theta_b = np.asarray(inputs["theta_b"], np.float32)
    phi_w = np.asarray(inputs["phi_w"], np.float32)
    W_w = np.asarray(inputs["W_w"], np.float32)
    W_b = np.asarray(inputs["W_b"], np.float32)

    a_t = np.ascontiguousarray(phi_w.T @ theta_w)            # [C, C]
    v = phi_w.T @ theta_b                                    # [C]
    gv = np.ascontiguousarray(np.concatenate(
        [v[:, None], g_w.T, np.zeros((C, 1), np.float32)], axis=1))
    b_out = W_w @ g_b + W_b                                  # [C]
    w_aug = np.ascontiguousarray(
        np.concatenate([W_w.T, b_out[None, :]], axis=0))     # [65, C]

    in_maps = []
    for core in range(NCORES):
        b, hh = core // 2, core % 2
        x0f = x0[b].reshape(C, N)
        x1f = x1[b].reshape(C, N)
        in_maps.append({
            "x0": x0f,
            "x1h": np.ascontiguousarray(x1f[:, hh * QH:(hh + 1) * QH]),
            "res": np.ascontiguousarray(x0f[:, hh * QH:(hh + 1) * QH]),
            "a_t": a_t,
            "gv": gv,
            "w_aug": w_aug,
            "eye": np.eye(C, dtype=np.float32),
        })
    return in_maps


def _run(inputs, trace=False):
    nc = _build()
    in_maps = _prep_in_maps(inputs)
    res = run_bass_kernel_spmd(nc, in_maps, core_ids=list(range(NCORES)),
                               trace=trace)
    out = np.empty((B, C, N), np.float32)
    for core in range(NCORES):
        b, hh = core // 2, core % 2
        out[b][:, hh * QH:(hh + 1) * QH] = res.results[core]["out"]
    return out.reshape(B, C, H, W), res


def kernel(**inputs) -> np.ndarray:
    out, _ = _run(inputs, trace=False)
    return out


# revision 20
# speedup vs baseline: 1.1994x; 1.0164x over previous
"""AdjustedNonLocalBlock on 8 TRN2 NeuronCores (float32r pipeline).

Math (per batch, N = H*W = 4096 positions):
    f = theta(x1)^T phi(x0);  P = softmax(f, axis=-1);
    y = P @ g(x0)^T;  out = W_w y^T + W_b + x0.

Reductions:
  - f[q,k] = x1[:,q]^T A x0[:,k] + t3[k] (+ per-q consts, dropped --
    softmax-invariant), A = theta_w^T phi_w, t3 = (phi_w^T theta_b)^T x0.
    t3 enters as the per-partition bias of the exp activation.
  - g's bias folds into the output bias b_out = W_w g_b + W_b (softmax
    rows sum to 1); the 1/Z normalization commutes with W_w (per-query
    diagonal), applied between the attention and projection matmuls.
  - Z comes free as a ones-column appended to g in the second matmul.

Sharding: core i = (batch i//2, query half i%2); softmax is over keys,
fully core-local -> no collectives.

Per-core dataflow (float32r matmuls, fp32 PSUM):
  U = A @ X0 [128,4096]; gaug per key tile kt: [t3 | g_raw^T (64) | 1].
  For each 1024-query pair qp (2 of them):
    for kt (32): S[128k,1024q] = U_kt^T X1_qp (two [128,512] matmuls)
      E = exp(S + t3_kt) [128,1024] on ScalarE (per-partition bias)
      Ya/Yb[65,512] += [g|1]^T E-half (two matmuls, separate banks).
    Z = Y[64]; y = Y[0:64] * (1/Z broadcast by a ones-matmul);
    out = W_aug @ [y; 1] + x0_res.
  U/gaug production is emitted just-in-time inside the first qp pass;
  the qp0 epilogues run inside qp1's loop (the DVE reciprocal runs
  early, the TensorE part late) so the PE never stalls on them; only
  the last pair of epilogues is exposed in the tail.
"""

import numpy as np

import concourse.bacc as bacc
import concourse.mybir as mybir
import concourse.tile as tile
from concourse.bass_utils import run_bass_kernel_spmd

B, C, CI = 4, 128, 64
H, W = 64, 64
N = H * W              # 4096
NCORES = 8
QH = N // 2            # 2048 queries per core
KT = N // 128          # 32 key tiles of 128
GSTR = 66              # gaug per-tile layout: [t3 | g (64) | ones]

F32 = mybir.dt.float32
F32R = mybir.dt.float32r

_CACHE = {}


def _f32(ap):
    return ap.bitcast(F32)


def _build():
    if "nc" in _CACHE:
        return _CACHE["nc"]

    nc = bacc.Bacc("TRN2", target_bir_lowering=False, debug=False,
                   num_devices=NCORES)
    x0_ext = nc.declare_dram_parameter("x0", [C, N], F32R, isOutput=False)
    x1_ext = nc.declare_dram_parameter("x1h", [C, QH], F32R, isOutput=False)
    res_ext = nc.declare_dram_parameter("res", [C, QH], F32, isOutput=False)
    at_ext = nc.declare_dram_parameter("a_t", [C, C], F32R, isOutput=False)
    gv_ext = nc.declare_dram_parameter("gv", [C, CI + 2], F32R,
                                       isOutput=False)
    wa_ext = nc.declare_dram_parameter("w_aug", [CI + 1, C], F32R,
                                       isOutput=False)
    eye_ext = nc.declare_dram_parameter("eye", [C, C], F32, isOutput=False)
    out_ext = nc.declare_dram_parameter("out", [C, QH], F32, isOutput=True)

    AF = mybir.ActivationFunctionType

    with tile.TileContext(nc) as tc:
        with (
            tc.tile_pool(name="const", bufs=1) as constp,
            tc.tile_pool(name="data", bufs=1) as datap,
            tc.tile_pool(name="epool", bufs=4) as epool,
            tc.tile_pool(name="spool", bufs=2, space="PSUM") as spool,
            tc.tile_pool(name="ypool", bufs=2, space="PSUM") as ypool,
            tc.tile_pool(name="smallp", bufs=2, space="PSUM") as smallp,
            tc.tile_pool(name="ysbp", bufs=2) as ysbp,
            tc.tile_pool(name="outp", bufs=3) as outp,
            tc.tile_pool(name="rzp", bufs=2) as rzp,
        ):
            # table preload: a tiny Exp warms the exp table set while
            # the input DMAs are still in flight
            scr = constp.tile([1, 2], F32)
            nc.vector.memset(scr[:], 1.0)
            nc.scalar.activation(scr[0:1, 1:2], scr[0:1, 0:1], AF.Exp)

            ones_sb = constp.tile([1, CI], F32)
            nc.vector.memset(ones_sb[:], 1.0)

            # small inputs first, then the big ones in chunks so the
            # first compute can start after ~1 chunk
            at_sb = constp.tile([C, C], F32R)
            nc.sync.dma_start(at_sb[:], at_ext[:])
            x0_sb = datap.tile([C, N], F32R)
            x1_sb = datap.tile([C, QH], F32R)
            nc.sync.dma_start(x0_sb[:, 0:512], x0_ext[:, 0:512])
            gv_sb = constp.tile([C, CI + 2], F32R)
            nc.sync.dma_start(gv_sb[:], gv_ext[:])
            nc.sync.dma_start(x1_sb[:, 0:512], x1_ext[:, 0:512])
            nc.sync.dma_start(x1_sb[:, 512:1024], x1_ext[:, 512:1024])
            nc.sync.dma_start(x0_sb[:, 512:1024], x0_ext[:, 512:1024])
            nc.sync.dma_start(x0_sb[:, 1024:2048], x0_ext[:, 1024:2048])
            nc.sync.dma_start(x0_sb[:, 2048:4096], x0_ext[:, 2048:4096])
            wa_sb = constp.tile([CI + 1, C], F32R)
            nc.sync.dma_start(wa_sb[:], wa_ext[:])
            eye_sb = constp.tile([C, C], F32)
            nc.sync.dma_start(eye_sb[:], eye_ext[:])
            id1_sb = constp.tile([1, 1], F32)
            nc.vector.memset(id1_sb[:], 1.0)
            nc.sync.dma_start(x1_sb[:, 1024:2048], x1_ext[:, 1024:2048])
            res_sb = datap.tile([C, QH], F32)

            U_sb = datap.tile([C, N], F32R)
            gaug_sb = datap.tile([C, KT * GSTR], F32R)
            nc.vector.memset(_f32(gaug_sb[:]), 1.0)  # preset ones cols
            yaug_sb = datap.tile([CI + 1, QH], F32R)
            nc.vector.memset(_f32(yaug_sb[CI:CI + 1, :]), 1.0)

            def emit_u_chunk(c):
                pu = smallp.tile([C, 512], F32, tag="sm")
                nc.tensor.matmul(pu[:], at_sb[:],
                                 x0_sb[:, c * 512:(c + 1) * 512],
                                 start=True, stop=True)
                nc.vector.tensor_copy(U_sb[:, c * 512:(c + 1) * 512], pu[:])

            def emit_gaug(kt):
                # [t3 | g_raw^T] -> cols 0..64 of this tile's stripe
                pg = smallp.tile([C, 512], F32, tag="sm")
                nc.tensor.matmul(pg[:, 0:CI + 2],
                                 x0_sb[:, kt * 128:(kt + 1) * 128],
                                 gv_sb[:], start=True, stop=True)
                nc.vector.tensor_copy(
                    gaug_sb[:, kt * GSTR:kt * GSTR + CI + 1], pg[:, 0:CI + 1])

            # prologue pieces needed before the first S tile / mm2
            emit_u_chunk(0)
            emit_gaug(0)
            emit_gaug(1)

            def emit_mm1(qp, kt):
                s = spool.tile([C, 1024], F32)
                q0 = qp * 1024
                lhs = U_sb[:, kt * 128:(kt + 1) * 128]
                nc.tensor.matmul(s[:, 0:512], lhs, x1_sb[:, q0:q0 + 512],
                                 start=True, stop=True)
                nc.tensor.matmul(s[:, 512:1024], lhs,
                                 x1_sb[:, q0 + 512:q0 + 1024],
                                 start=True, stop=True)
                return s

            def emit_epilogue_fronts(items):
                # Per Y: copy (frees the bank), then reciprocal of Z,
                # then GPSIMD broadcasts 1/Z across partitions (keeps
                # both the PE and the mul off the 1/Z critical path).
                ret = []
                for qc, Y in items:
                    ysb = ysbp.tile([CI, 512], F32)
                    nc.vector.tensor_copy(ysb[:], Y[0:CI, :])
                    zrow = rzp.tile([1, 512], F32, tag="zrow")
                    nc.vector.tensor_copy(zrow[:], Y[CI:CI + 1, :])
                    rz = rzp.tile([1, 512], F32)
                    nc.vector.reciprocal(rz[:], zrow[:])
                    bcs = ysbp.tile([CI, 512], F32, tag="bcs")
                    nc.gpsimd.partition_broadcast(bcs[:], rz[:], channels=CI)
                    ret.append((qc, ysb, bcs))
                return ret

            def emit_epilogue_fronts_tail(items):
                # Tail variant: 1/Z computed on Z transposed across 128
                # partitions (PE transposes are free in the tail; the
                # [1,512] DVE reciprocal would cost 3.3us each).
                pre = []
                for qc, Y in items:
                    ysb = ysbp.tile([CI, 512], F32)
                    nc.vector.tensor_copy(ysb[:], Y[0:CI, :])
                    zrow = rzp.tile([1, 512], F32, tag="zrow")
                    nc.vector.tensor_copy(zrow[:], Y[CI:CI + 1, :])
                    pre.append((qc, ysb, zrow))
                ret = []
                for qc, ysb, zrow in pre:
                    zt = smallp.tile([C, 512], F32, tag="sm")
                    for j in range(4):
                        nc.tensor.transpose(
                            zt[:, j:j + 1],
                            zrow[0:1, j * 128:(j + 1) * 128], id1_sb[:])
                    rzt = rzp.tile([C, 4], F32, tag="rzt")
                    nc.vector.reciprocal(rzt[:], zt[:, 0:4])
                    tr2 = smallp.tile([C, 512], F32, tag="sm")
                    for j in range(4):
                        nc.tensor.transpose(tr2[0:1, j * 128:(j + 1) * 128],
                                            rzt[:, j:j + 1], eye_sb[:])
                    rz = rzp.tile([1, 512], F32)
                    nc.vector.tensor_copy(rz[:], tr2[0:1, 0:512])
                    bcs = ysbp.tile([CI, 512], F32, tag="bcs")
                    nc.gpsimd.partition_broadcast(bcs[:], rz[:], channels=CI)
                    ret.append((qc, ysb, bcs))
                return ret

            def emit_epilogue_back(qc, ysb, bcs, anchor=None):
                q0 = qc * 512
                nc.vector.tensor_mul(yaug_sb[0:CI, q0:q0 + 512],
                                     ysb[:], bcs[:])
                pr = smallp.tile([C, 512], F32, tag="sm")
                prj = nc.tensor.matmul(pr[:], wa_sb[:],
                                       yaug_sb[:, q0:q0 + 512],
                                       start=True, stop=True)
                if anchor is not None:
                    # pin the projection behind a late matmul so the
                    # scheduler cannot hoist it into a stall
                    tile.add_dep_helper(prj.ins, anchor.ins, False,
                                        "defer epilogue proj")
                ot = outp.tile([C, 512], F32)
                nc.vector.tensor_add(ot[:], pr[:], res_sb[:, q0:q0 + 512])
                nc.sync.dma_start(out_ext[:, q0:q0 + 512], ot[:])

            fronts = []  # epilogue fronts whose TensorE part is pending
            for qp in range(2):
                ya = ypool.tile([CI + 1, 512], F32, tag="y")
                yb = ypool.tile([CI + 1, 512], F32, tag="y")
                s_cur = emit_mm1(qp, 0)
                for kt in range(KT):
                    e = epool.tile([C, 1024], F32R)
                    nc.scalar.activation(
                        e[:], s_cur[:], AF.Exp,
                        bias=_f32(gaug_sb[:, kt * GSTR:kt * GSTR + 1]))
                    if qp == 0:
                        # just-in-time prologue during the first pass
                        if kt % 4 == 0 and kt // 4 + 1 < 8:
                            emit_u_chunk(kt // 4 + 1)
                        if kt + 2 < KT:
                            emit_gaug(kt + 2)
                        if kt == 0:
                            nc.sync.dma_start(res_sb[:], res_ext[:])
                    else:
                        # TensorE part of qp0's epilogues, far enough in
                        # that the reciprocal results are long ready
                        if kt in (10, 12) and fronts:
                            emit_epilogue_back(*fronts.pop(0),
                                               anchor=prev_mm2)
                    if kt + 1 < KT:
                        s_cur = emit_mm1(qp, kt + 1)
                    elif qp == 0:
                        s_cur = emit_mm1(1, 0)
                    st, sp = kt == 0, kt == KT - 1
                    glhs = gaug_sb[:, kt * GSTR + 1:kt * GSTR + GSTR]
                    prev_mm2 = nc.tensor.matmul(ya[:], glhs, e[:, 0:512],
                                                start=st, stop=sp)
                    nc.tensor.matmul(yb[:], glhs, e[:, 512:1024],
                                     start=st, stop=sp)
                # DVE fronts run now (free the Y banks for the next qp)
                if qp == 0:
                    fronts.extend(emit_epilogue_fronts(
                        [(2 * qp, ya), (2 * qp + 1, yb)]))
                else:
                    fronts.extend(emit_epilogue_fronts_tail(
                        [(2 * qp, ya), (2 * qp + 1, yb)]))

            # exposed tail: fast 1/Z via PE transposes (PE is idle),
            # then the usual backs
            tails = []
            for qc, ysb, bcs in fronts:
                tails.append((qc, ysb, bcs))
            fronts.clear()
            emit_epilogue_back(*tails.pop(0))
            emit_epilogue_back(*tails.pop(0))

    nc.compile()
    _CACHE["nc"] = nc
    return nc


def _prep_in_maps(inputs):
    x0 = np.ascontiguousarray(np.asarray(inputs["x0"], np.float32))
    x1 = np.ascontiguousarray(np.asarray(inputs["x1"], np.float32))
    g_w = np.asarray(inputs["g_w"], np.float32)
    g_b = np.asarray(inputs["g_b"], np.float32)
    theta_w = np.asarray(inputs["theta_w"], np.float32)
    theta_b = np.asarray(inputs["theta_b"], np.float32)
    phi_w = np.asarray(inputs["phi_w"], np.float32)
    W_w = np.asarray(inputs["W_w"], np.float32)
    W_b = np.asarray(inputs["W_b"], np.float32)

    a_t = np.ascontiguousarray(phi_w.T @ theta_w)            # [C, C]
    v = phi_w.T @ theta_b                                    # [C]
    gv = np.ascontiguousarray(np.concatenate(
        [v[:, None], g_w.T, np.zeros((C, 1), np.float32)], axis=1))
    b_out = W_w @ g_b + W_b                                  # [C]
    w_aug = np.ascontiguousarray(
        np.concatenate([W_w.T, b_out[None, :]], axis=0))     # [65, C]

    in_maps = []
    for core in range(NCORES):
        b, hh = core // 2, core % 2
        x0f = x0[b].reshape(C, N)
        x1f = x1[b].reshape(C, N)
        in_maps.append({
            "x0": x0f,
            "x1h": np.ascontiguousarray(x1f[:, hh * QH:(hh + 1) * QH]),
            "res": np.ascontiguousarray(x0f[:, hh * QH:(hh + 1) * QH]),
            "a_t": a_t,
            "gv": gv,
            "w_aug": w_aug,
            "eye": np.eye(C, dtype=np.float32),
        })
    return in_maps


def _run(inputs, trace=False):
    nc = _build()
    in_maps = _prep_in_maps(inputs)
    res = run_bass_kernel_spmd(nc, in_maps, core_ids=list(range(NCORES)),
                               trace=trace)
    out = np.empty((B, C, N), np.float32)
    for core in range(NCORES):
        b, hh = core // 2, core % 2
        out[b][:, hh * QH:(hh + 1) * QH] = res.results[core]["out"]
    return out.reshape(B, C, H, W), res


def kernel(**inputs) -> np.ndarray:
    out, _ = _run(inputs, trace=False)
    return out
